# revision 13
# baseline (speedup 1.0000x reference)
"""Trainium2 Bass kernel for nn_Attention (Bahdanau attention + LSTM decoder scan).

Data-parallel over batch B=512 across 8 NeuronCores (64 rows/core, no
collectives).  Per core:
  hoist:  pHT[h,(t,b)] = i2h_w @ batch_H^T   (PE, written to DRAM, bf16)
  scan (26 steps):
    phT   = h2h_w @ h^T                       (PE)
    z     = pHT + broadcast(phT); T = tanh(z) (DVE 2x + ACT), streamed from DRAM
    acc   = sum_c w_c * T_c                   (DVE scalar_tensor_tensor chain)
    e^T   = ones^T-reduce over 128 h-rem      (PE, acc blocks as weights)
    alpha = softmax_t(e)                      (PE transposes + ACT exp + DVE)
    ctx^T = bHC-tiles(weights) @ alpha_b      (PE, per-b matvec)
    gates = [ctx;h]^T-chunks @ Wg + tok_bias  (PE, Wg streamed from DRAM)
    LSTM pointwise (sigmoid via tanh table)   (ACT+DVE)
    logits^T = gen^T-tiles @ h^T              (PE) -> DRAM out
All matmul operands bf16 (f32 PSUM accum); c-state f32.
"""
import sys
from contextlib import ExitStack

import numpy as np
import ml_dtypes

sys.path.insert(0, "/opt/trn_rl_repo")

import concourse.bass as bass  # noqa: E402
from concourse import bacc  # noqa: E402
import concourse.tile as tile  # noqa: E402
from concourse import mybir  # noqa: E402
from concourse.bass_utils import run_bass_kernel_spmd  # noqa: E402

BF16 = ml_dtypes.bfloat16
LAST_RESULT = None
BF = mybir.dt.bfloat16
F32 = mybir.dt.float32
ADD = mybir.AluOpType.add
MULT = mybir.AluOpType.mult
TANH = mybir.ActivationFunctionType.Tanh
EXP = mybir.ActivationFunctionType.Exp

B, T, D, H, V, STEPS = 512, 128, 1024, 1024, 256, 26
NC_ = 8
BL = B // NC_          # 64 batch rows per core
TB = T * BL            # 8192 (t,b) columns, t outer / b inner
FB = 2048              # free-block of (t,b) columns
NF = TB // FB          # 4
TFB = FB // BL         # 32 t-values per free block
HC = 8                 # 128-chunks of H
DC = 8                 # 128-chunks of D
KG = 16                # gates K chunks (ctx 8 + h 8)
G4 = 4 * H             # 4096


def build_kernel(steps=STEPS, skip=frozenset()):
    nc = bacc.Bacc("TRN2", target_bir_lowering=False, debug=False)
    # inputs
    bHT = nc.declare_dram_parameter("bHT", [D, TB], BF, isOutput=False)
    bHC = nc.declare_dram_parameter("bHC", [T, BL * D], BF, isOutput=False)
    i2hT = nc.declare_dram_parameter("i2hT", [D, H], BF, isOutput=False)
    h2hT = nc.declare_dram_parameter("h2hT", [H, H], BF, isOutput=False)
    wsc = nc.declare_dram_parameter("wsc", [128, HC], F32, isOutput=False)
    wscb = nc.declare_dram_parameter("wscb", [128, HC], BF, isOutput=False)
    h2hbT = nc.declare_dram_parameter("h2hbT", [128, HC], F32, isOutput=False)
    WgT = nc.declare_dram_parameter("WgT", [2 * H, G4], BF, isOutput=False)
    tb = nc.declare_dram_parameter("tb", [STEPS, BL, G4], BF, isOutput=False)
    genT = nc.declare_dram_parameter("genT", [H, V], BF, isOutput=False)
    genbT = nc.declare_dram_parameter("genbT", [128, 2], F32, isOutput=False)
    ident = nc.declare_dram_parameter("ident", [128, 128], BF, isOutput=False)
    onesc = nc.declare_dram_parameter("onesc", [128, 1], BF, isOutput=False)
    out = nc.declare_dram_parameter("out", [STEPS, 2, 128, BL], F32, isOutput=True)
    # internal scratch in DRAM
    pHT = nc.dram_tensor("pHT", [HC, 128, TB], BF)

    ctx = ExitStack()
    tc = ctx.enter_context(tile.TileContext(nc))

    # ---------------- persistent tiles ----------------
    res = ctx.enter_context(tc.tile_pool(name="res", bufs=1))
    bHC_sb = res.tile([T, BL * D], BF, name="bHC_sb")        # 128 KiB/part
    h2hT_sb = res.tile([128, HC * H], BF, name="h2hT_sb")    # 16 KiB/part
    wsc_sb = res.tile([128, HC], F32, name="wsc_sb")
    wscb_sb = res.tile([128, HC], BF, name="wscb_sb")
    h2hbT_sb = res.tile([128, HC], F32, name="h2hbT_sb")
    genbT_sb = res.tile([128, 2], F32, name="genbT_sb")
    ident_sb = res.tile([128, 128], BF, name="ident_sb")
    ones_sb = res.tile([128, 1], BF, name="ones_sb")
    hT_sb = res.tile([128, HC * BL], BF, name="hT_sb")       # h^T chunks [h,b]
    cB_sb = res.tile([BL, H], F32, name="cB_sb")             # c state (b-layout)
    phT_sb = res.tile([128, HC * BL], BF, name="phT_sb")
    ctxT_sb = res.tile([128, DC * BL], BF, name="ctxT_sb")
    gates_sb = res.tile([BL, G4], BF, name="gates_sb")
    genT_sb = res.tile([128, HC * V], BF, name="genT_sb")

    nc.sync.dma_start(bHC_sb[:], bHC[:])
    for k in range(HC):
        nc.sync.dma_start(h2hT_sb[:, k * H:(k + 1) * H], h2hT[k * 128:(k + 1) * 128, :])
    nc.sync.dma_start(wsc_sb[:], wsc[:])
    nc.sync.dma_start(wscb_sb[:], wscb[:])
    nc.sync.dma_start(h2hbT_sb[:], h2hbT[:])
    nc.sync.dma_start(genbT_sb[:], genbT[:])
    nc.sync.dma_start(ident_sb[:], ident[:])
    nc.sync.dma_start(ones_sb[:], onesc[:])
    for k in range(HC):
        nc.sync.dma_start(genT_sb[:, k * V:(k + 1) * V],
                          genT[k * 128:(k + 1) * 128, :])
    nc.vector.memset(hT_sb[:], 0.0)
    nc.vector.memset(cB_sb[:], 0.0)

    # ---------------- hoist: pHT = i2h @ bH^T ----------------
    with tc.tile_pool(name="hoist", bufs=1) as hres, \
         tc.tile_pool(name="hrhs", bufs=2) as hrhs, \
         tc.tile_pool(name="hst", bufs=3) as hst, \
         tc.tile_pool(name="hps", bufs=4, space="PSUM") as hps:
        i2hT_sb = hres.tile([128, HC * H], BF, name="i2hT_sb")
        for k in range(DC):
            nc.sync.dma_start(i2hT_sb[:, k * H:(k + 1) * H],
                              i2hT[k * 128:(k + 1) * 128, :])
        for n in range(TB // 512):
            rt = hrhs.tile([128, DC * 512], BF, name="hoist_rhs", tag="hrhs")
            for k in range(DC):
                nc.sync.dma_start(rt[:, k * 512:(k + 1) * 512],
                                  bHT[k * 128:(k + 1) * 128, n * 512:(n + 1) * 512])
            for c in range(HC):
                ps = hps.tile([128, 512], F32, name="hoist_ps")
                for k in range(DC):
                    nc.tensor.matmul(ps[:], i2hT_sb[:, k * H + c * 128: k * H + (c + 1) * 128],
                                     rt[:, k * 512:(k + 1) * 512],
                                     start=(k == 0), stop=(k == DC - 1))
                st = hst.tile([128, 512], BF, name="hoist_stage")
                nc.vector.tensor_copy(st[:], ps[:])
                nc.gpsimd.dma_start(pHT[c, :, n * 512:(n + 1) * 512], st[:])

    # ---------------- scan pools ----------------
    phs = ctx.enter_context(tc.tile_pool(name="phs", bufs=3))       # pH stream
    zp = ctx.enter_context(tc.tile_pool(name="zp", bufs=2))         # z tiles
    accp = ctx.enter_context(tc.tile_pool(name="accp", bufs=2))     # chain acc
    wgp = ctx.enter_context(tc.tile_pool(name="wgp", bufs=3))       # Wg stream
    tbp = ctx.enter_context(tc.tile_pool(name="tbp", bufs=1))       # tok bias
    smx = ctx.enter_context(tc.tile_pool(name="smx", bufs=1))       # softmax bits
    lst = ctx.enter_context(tc.tile_pool(name="lst", bufs=1))       # LSTM tmp
    lout = ctx.enter_context(tc.tile_pool(name="lout", bufs=2))     # logits stage

    ph_ps = ctx.enter_context(tc.tile_pool(name="ph_ps", bufs=2, space="PSUM"))
    e_ps = ctx.enter_context(tc.tile_pool(name="e_ps", bufs=1, space="PSUM"))
    tr_ps = ph_ps
    cx_ps = ctx.enter_context(tc.tile_pool(name="cx_ps", bufs=1, space="PSUM"))
    g_ps = ctx.enter_context(tc.tile_pool(name="g_ps", bufs=1, space="PSUM"))
    ht_ps = ph_ps
    lo_ps = ph_ps

    for s in range(steps):
        # ---- 1. phT = h2h @ h^T + bias ----
        for c in range(HC if 'ph' not in skip else 0):
            pp = ph_ps.tile([128, BL], F32, name="ph_ps_t", tag="tmp")
            for k in range(HC):
                nc.tensor.matmul(pp[:], h2hT_sb[:, k * H + c * 128: k * H + (c + 1) * 128],
                                 hT_sb[:, k * BL:(k + 1) * BL],
                                 start=(k == 0), stop=(k == HC - 1))
            nc.scalar.activation(phT_sb[:, c * BL:(c + 1) * BL], pp[:],
                                 mybir.ActivationFunctionType.Copy,
                                 bias=0.0, scale=1.0)
        # ---- 2. z/tanh/chain + per-f score reduce ----
        ep = e_ps.tile([128, BL], F32, name="eT_ps")
        for f in range(NF if 'att' not in skip else 0):
            acc = accp.tile([128, FB], BF, name="acc_t")
            for c in range(HC):
                pht = phs.tile([128, FB], BF, name="pH_t")
                nc.gpsimd.dma_start(pht[:], pHT[c, :, f * FB:(f + 1) * FB])
                z = zp.tile([128, FB], BF, name="z_t")
                z3 = z[:].rearrange("p (t b) -> p t b", b=BL)
                ph3 = pht[:].rearrange("p (t b) -> p t b", b=BL)
                phb = phT_sb[:, c * BL:(c + 1) * BL].unsqueeze(1).broadcast_to(
                    (128, TFB, BL))
                nc.vector.tensor_tensor(z3, ph3, phb, ADD)
                nc.scalar.activation(z[:], z[:], TANH)
                if c == 0:
                    nc.vector.tensor_scalar(acc[:], z[:], wsc_sb[:, 0:1], None, MULT)
                elif c < HC - 1:
                    nc.vector.tensor_scalar(z[:], z[:], wsc_sb[:, c:c + 1], None, MULT)
                    nc.vector.tensor_tensor(acc[:], acc[:], z[:], ADD)
                else:
                    z_last = z
            # score reduce rows t in [f*TFB, (f+1)*TFB): strided lhsT per b
            a3 = acc[:].rearrange("p (t b) -> p b t", b=BL)
            zl3 = z_last[:].rearrange("p (t b) -> p b t", b=BL)
            for bb in range(BL if 'score' not in skip else 0):
                nc.tensor.matmul(ep[f * TFB:(f + 1) * TFB, bb:bb + 1],
                                 a3[:, bb, :], ones_sb[:],
                                 start=True, stop=False,
                                 tile_position=(0, f * TFB))
                nc.tensor.matmul(ep[f * TFB:(f + 1) * TFB, bb:bb + 1],
                                 zl3[:, bb, :], wscb_sb[:, HC - 1:HC],
                                 start=False, stop=True,
                                 tile_position=(0, f * TFB))
        # ---- 4. softmax over t ----
        e1 = smx.tile([128, BL], BF, name="e1")
        nc.scalar.copy(e1[:], ep[:])
        tp = tr_ps.tile([128, 128], BF, name="tr_t", tag="tmp")[0:BL, :]
        nc.tensor.transpose(tp[:], e1[:], ident_sb[:])
        ex = smx.tile([BL, 128], F32, name="ex")
        nc.scalar.activation(ex[:], tp[:], EXP)
        sg = smx.tile([BL, 1], F32, name="sig")
        nc.vector.tensor_reduce(sg[:], ex[:], mybir.AxisListType.X, ADD)
        rc = smx.tile([BL, 1], F32, name="rec")
        nc.vector.reciprocal(rc[:], sg[:])
        al = smx.tile([BL, 128], BF, name="alpha")
        nc.vector.tensor_scalar(al[:], ex[:], rc[:], None, MULT)
        ap2 = tr_ps.tile([128, 128], BF, name="tr_t2", tag="tmp")[:, 0:BL]
        nc.tensor.transpose(ap2[:], al[:], ident_sb[0:BL, 0:BL])
        alT = smx.tile([128, BL], BF, name="alT")
        nc.scalar.copy(alT[:], ap2[:])
        # ---- 5. ctx^T[d, b] ----
        cxp = cx_ps.tile([128, DC * BL], F32, name="ctxT_ps")
        for c in range(DC if 'ctx' not in skip else 0):
            for bb in range(BL):
                nc.tensor.matmul(cxp[:, c * BL + bb: c * BL + bb + 1],
                                 bHC_sb[:, bb * D + c * 128: bb * D + (c + 1) * 128],
                                 alT[:, bb:bb + 1], start=True, stop=True)
            nc.scalar.copy(ctxT_sb[:, c * BL:(c + 1) * BL],
                           cxp[:, c * BL:(c + 1) * BL])
        # ---- 6. gates = [ctx; h]^T @ Wg + tb ----
        for half in range(2 if 'gates' not in skip else 0):
            gps = [g_ps.tile([BL, 512], F32, name=f"g_ps_{j}", tag=f"g{j}")
                   for j in range(4)]
            for k in list(range(DC, KG)) + list(range(DC)):
                wg = wgp.tile([128, 2048], BF, name="wg_t")
                nc.sync.dma_start(wg[:], WgT[k * 128:(k + 1) * 128,
                                             half * 2048:(half + 1) * 2048])
                xk = (ctxT_sb[:, k * BL:(k + 1) * BL] if k < DC
                      else hT_sb[:, (k - DC) * BL:(k - DC + 1) * BL])
                for j in range(4):
                    nc.tensor.matmul(gps[j][:], xk, wg[:, j * 512:(j + 1) * 512],
                                     start=(k == DC), stop=(k == DC - 1))
            for jj in range(2):
                tbt = tbp.tile([BL, 1024], BF, name="tb_t")
                nc.gpsimd.dma_start(tbt[:], tb[s, :, half * 2048 + jj * 1024:
                                              half * 2048 + (jj + 1) * 1024])
                for j2 in range(2):
                    j = jj * 2 + j2
                    nb = half * 4 + j
                    nc.vector.scalar_tensor_tensor(
                        gates_sb[:, nb * 512:(nb + 1) * 512], gps[j][:], 1.0,
                        tbt[:, j2 * 512:(j2 + 1) * 512], MULT, ADD)
        # ---- 7. LSTM pointwise (sigmoid via tanh), in-place on gates ----
        sif = gates_sb[:, 0:2 * H]
        so = gates_sb[:, 3 * H:4 * H]
        tg = gates_sb[:, 2 * H:3 * H]
        nc.scalar.activation(sif, sif, TANH, scale=0.5)
        nc.scalar.activation(so, so, TANH, scale=0.5)
        nc.scalar.activation(tg, tg, TANH)
        nc.vector.tensor_scalar(sif, sif, 0.5, 0.5, MULT, ADD)
        nc.vector.tensor_scalar(so, so, 0.5, 0.5, MULT, ADD)
        m1 = gates_sb[:, 0:H]
        nc.vector.tensor_tensor(m1, m1, tg, MULT)
        nc.vector.tensor_tensor(cB_sb[:], gates_sb[:, H:2 * H], cB_sb[:], MULT)
        nc.vector.tensor_tensor(cB_sb[:], cB_sb[:], m1, ADD)
        th = gates_sb[:, 2 * H:3 * H]
        nc.scalar.activation(th, cB_sb[:], TANH)
        hB = gates_sb[:, 3 * H:4 * H]
        nc.vector.tensor_tensor(hB, so, th, MULT)
        # ---- 8. transpose h -> hT ----
        for k in range(HC):
            hp = ht_ps.tile([128, BL], BF, name="hT_ps_t", tag="tmp")
            nc.tensor.transpose(hp[:], hB[:, k * 128:(k + 1) * 128],
                                ident_sb[0:BL, 0:BL])
            nc.scalar.copy(hT_sb[:, k * BL:(k + 1) * BL], hp[:])
        # ---- 9. logits^T ----
        for vc in range(2):
            lp = lo_ps.tile([128, BL], F32, name="lo_ps_t", tag="tmp")
            for k in range(HC):
                nc.tensor.matmul(lp[:], genT_sb[:, k * V + vc * 128: k * V + (vc + 1) * 128],
                                 hT_sb[:, k * BL:(k + 1) * BL],
                                 start=(k == 0), stop=(k == HC - 1))
            lo = lout.tile([128, BL], F32, name="lo_st")
            nc.vector.tensor_scalar(lo[:], lp[:], genbT_sb[:, vc:vc + 1], None, ADD)
            nc.gpsimd.dma_start(out[s, vc], lo[:])

    ctx.close()
    nc.compile()
    return nc


def _prep_core(ci, batch_H, text, i2h_w, h2h_w, h2h_b, score_w, W_ih, W_hh,
               b_ih, b_hh, gen_w, gen_b, shared):
    bH = batch_H[ci * BL:(ci + 1) * BL]          # [64, 128, 1024] f32
    tx = text[ci * BL:(ci + 1) * BL]             # [64, 26]
    # bHT [d, (t,b)]: bHT[d, t*64+b] = bH[b, t, d]
    bHT = np.ascontiguousarray(bH.transpose(2, 1, 0).reshape(D, TB)).astype(BF16)
    # bHC [t, b*D+d]
    bHC = np.ascontiguousarray(bH.transpose(1, 0, 2).reshape(T, BL * D)).astype(BF16)
    # tok bias [26, 64, 4096]
    tbv = shared["Wtok"][:, tx.astype(np.int64)].transpose(2, 1, 0)  # [26,64,4096]
    tbv = np.ascontiguousarray(tbv).astype(BF16)
    m = dict(shared["const"])
    m.update({"bHT": bHT, "bHC": bHC, "tb": tbv})
    return m


def kernel(batch_H, text, i2h_w, h2h_w, h2h_b, score_w, W_ih, W_hh, b_ih, b_hh,
           gen_w, gen_b):
    batch_H = np.asarray(batch_H, dtype=np.float32)
    text = np.asarray(text)
    f32 = lambda x: np.asarray(x, dtype=np.float32)
    i2h_w, h2h_w, h2h_b = f32(i2h_w), f32(h2h_w), f32(h2h_b)
    score_w, W_ih, W_hh = f32(score_w), f32(W_ih), f32(W_hh)
    b_ih, b_hh, gen_w, gen_b = f32(b_ih), f32(b_hh), f32(gen_w), f32(gen_b)

    Wtok = (W_ih[:, D:] + (b_ih + b_hh)[:, None]).astype(np.float32)  # [4096, 256]
    const = {
        "i2hT": np.ascontiguousarray(i2h_w.T).astype(BF16),
        "h2hT": np.ascontiguousarray(h2h_w.T).astype(BF16),
        "wsc": np.ascontiguousarray(score_w[0].reshape(HC, 128).T).astype(np.float32),
        "wscb": np.ascontiguousarray(score_w[0].reshape(HC, 128).T).astype(BF16),
        "h2hbT": np.ascontiguousarray(h2h_b.reshape(HC, 128).T).astype(np.float32),
        "WgT": np.ascontiguousarray(
            np.concatenate([W_ih[:, :D], W_hh], axis=1).T).astype(BF16),
        "genT": np.ascontiguousarray(gen_w.T).astype(BF16),
        "genbT": np.ascontiguousarray(gen_b.reshape(2, 128).T).astype(np.float32),
        "ident": np.eye(128, dtype=BF16),
        "onesc": np.ones((128, 1), dtype=BF16),
    }
    shared = {"const": const, "Wtok": Wtok}

    nc = build_kernel()
    in_maps = [
        _prep_core(ci, batch_H, text, i2h_w, h2h_w, h2h_b, score_w, W_ih, W_hh,
                   b_ih, b_hh, gen_w, gen_b, shared)
        for ci in range(NC_)
    ]
    import os
    do_trace = bool(int(os.environ.get("KERNEL_TRACE", "0")))
    res = run_bass_kernel_spmd(nc, in_maps, core_ids=list(range(NC_)),
                               trace=do_trace)
    global LAST_RESULT
    LAST_RESULT = res
    outs = res.results  # list of dicts per core
    logits = np.zeros((B, STEPS, V), dtype=np.float32)
    for ci in range(NC_):
        o = outs[ci]["out"] if isinstance(outs[ci], dict) else outs[ci]
        # o [26, 2, 128, 64] -> logits[b, s, vc*128+p]
        logits[ci * BL:(ci + 1) * BL] = o.transpose(3, 0, 1, 2).reshape(BL, STEPS, V)
    return logits


if __name__ == "__main__":
    np.random.seed(0)
    import reference
    inp = {k: np.asarray(v) for k, v in reference.setup_inputs().items()}
    got = kernel(**inp)
    exp = np.asarray(reference.reference(**inp))
    l2 = np.linalg.norm(got - exp) / np.linalg.norm(exp)
    print("l2 rel err:", l2)



# revision 14
# speedup vs baseline: 1.0739x; 1.0739x over previous
"""Trainium2 Bass kernel for nn_Attention (Bahdanau attention + LSTM scan).

Data-parallel over batch B=512 across 8 NeuronCores (64 rows/core).  Per core
the 64 rows are split into two 32-row half-batch streams (A/B) pipelined so
one stream's softmax/ctx/gates/LSTM tail hides inside the other stream's tanh
window (the ACT engine, at ~0.83 ns/elem-row, is the throughput bound).

Precision: matmul operands may mix dtypes (PE upconverts); all "moving" rhs
operands (alpha, ctx, h, onehot) stay bf16, stationary weights use fp8-e3m4
(W_ih with an extra e3m4 residual term), bHC fp8-e3m4, h2h fp8-e4m3,
logits/genT bf16.  pH is bf16, streamed from DRAM each step (z-add runs at
DVE 2x only for 2-byte dtypes).
"""
import sys
from contextlib import ExitStack

import numpy as np
import ml_dtypes

sys.path.insert(0, "/opt/trn_rl_repo")

import concourse.bass as bass  # noqa: E402
from concourse import bacc  # noqa: E402
import concourse.tile as tile  # noqa: E402
from concourse import mybir  # noqa: E402
from concourse.bass_utils import run_bass_kernel_spmd  # noqa: E402

BF16 = ml_dtypes.bfloat16
E3M4 = ml_dtypes.float8_e3m4
E4M3 = ml_dtypes.float8_e4m3
LAST_RESULT = None
BF = mybir.dt.bfloat16
F32 = mybir.dt.float32
E3 = mybir.dt.float8e3
E4 = mybir.dt.float8e4
ADD = mybir.AluOpType.add
MULT = mybir.AluOpType.mult
TANH = mybir.ActivationFunctionType.Tanh
EXP = mybir.ActivationFunctionType.Exp

B, T, D, H, V, STEPS = 512, 128, 1024, 1024, 256, 26
NC_ = 8
BL = B // NC_          # 64 batch rows per core
HB = BL // 2           # 32 rows per half-batch stream
HC = 8                 # 128-chunks of H
DC = 8                 # 128-chunks of D
G4 = 4 * H             # 4096
GRAN = T * HB          # 4096 cols per attention granule (t outer, b' inner)
SW = 128.0             # weight pre-scale before e4m3 quantization
SBHC = 16.0            # bHC pre-scale


def build_kernel(steps=STEPS):
    nc = bacc.Bacc("TRN2", target_bir_lowering=False, debug=False)
    # ---- inputs ----
    bHT = nc.declare_dram_parameter("bHT", [D, 2 * GRAN], BF, isOutput=False)
    bHC = nc.declare_dram_parameter("bHC", [T, BL * D], BF, isOutput=False)
    i2hT = nc.declare_dram_parameter("i2hT", [D, H], BF, isOutput=False)
    WihHi = nc.declare_dram_parameter("WihHi", [128, DC * G4], E4, isOutput=False)
    WihLo = nc.declare_dram_parameter("WihLo", [128, DC * G4], E4, isOutput=False)
    WhhHi = nc.declare_dram_parameter("WhhHi", [128, HC * G4], E4, isOutput=False)
    WhhLo = nc.declare_dram_parameter("WhhLo", [128, HC * G4], E4, isOutput=False)
    WtokT = nc.declare_dram_parameter("WtokT", [128, 2 * G4], E4, isOutput=False)
    h2hT = nc.declare_dram_parameter("h2hT", [128, HC * H], E4, isOutput=False)
    h2hbT = nc.declare_dram_parameter("h2hbT", [128, HC], F32, isOutput=False)
    genT = nc.declare_dram_parameter("genT", [128, HC * V], BF, isOutput=False)
    genb = nc.declare_dram_parameter("genb", [1, V], BF, isOutput=False)
    wscb = nc.declare_dram_parameter("wscb", [128, HC], BF, isOutput=False)
    oneh = nc.declare_dram_parameter("oneh", [STEPS, 2, 128, 2 * HB], BF, isOutput=False)
    ident = nc.declare_dram_parameter("ident", [128, 128], BF, isOutput=False)
    onesr = nc.declare_dram_parameter("onesr", [1, HB], BF, isOutput=False)
    out = nc.declare_dram_parameter("out", [STEPS, 2, 128, 2 * HB], F32, isOutput=True)
    pHT = nc.dram_tensor("pHT", [HC, 2, 128, GRAN], BF)

    ctx = ExitStack()
    tc = ctx.enter_context(tile.TileContext(nc))

    # ---------------- persistent SBUF ----------------
    res = ctx.enter_context(tc.tile_pool(name="res", bufs=1))
    WihHi_sb = res.tile([128, DC * G4], E4, name="WihHi_sb")   # 32 KiB/part
    WihLo_sb = res.tile([128, DC * G4], E4, name="WihLo_sb")   # 32 KiB
    WhhHi_sb = res.tile([128, HC * G4], E4, name="WhhHi_sb")   # 32 KiB
    WhhLo_sb = res.tile([128, HC * G4], E4, name="WhhLo_sb")   # 32 KiB
    Wtok_sb = res.tile([128, 2 * G4], E4, name="Wtok_sb")      # 8 KiB
    h2h_sb = res.tile([128, HC * H], E4, name="h2h_sb")        # 8 KiB
    h2hbT_sb = res.tile([128, HC], F32, name="h2hbT_sb")
    genT_sb = res.tile([128, HC * V], BF, name="genT_sb")      # 4 KiB
    genb_sb = res.tile([1, V], BF, name="genb_sb")
    wscb_sb = res.tile([128, HC], BF, name="wscb_sb")
    ident_sb = res.tile([128, 128], BF, name="ident_sb")
    onesr_sb = res.tile([1, HB], BF, name="onesr_sb")
    # per-stream state: layout [128, (hc 8, b' 32)]
    phT_sb = [res.tile([128, HC * HB], BF, name=f"phT{h}") for h in range(2)]
    hT_sb = [res.tile([128, HC * HB], BF, name=f"hT{h}") for h in range(2)]
    cB_sb = [res.tile([128, HC * HB], F32, name=f"cB{h}") for h in range(2)]
    ctxB_sb = [res.tile([128, DC * HB], BF, name=f"ctxB{h}") for h in range(2)]
    alB_sb = [res.tile([128, HB], BF, name=f"alB{h}") for h in range(2)]
    eS_sb = [res.tile([128, HB], F32, name=f"eS{h}") for h in range(2)]

    nc.sync.dma_start(WihHi_sb[:], WihHi[:])
    nc.sync.dma_start(WihLo_sb[:], WihLo[:])
    nc.sync.dma_start(WhhHi_sb[:], WhhHi[:])
    nc.sync.dma_start(WhhLo_sb[:], WhhLo[:])
    nc.sync.dma_start(Wtok_sb[:], WtokT[:])
    nc.sync.dma_start(h2h_sb[:], h2hT[:])
    nc.sync.dma_start(h2hbT_sb[:], h2hbT[:])
    nc.sync.dma_start(genT_sb[:], genT[:])
    nc.sync.dma_start(genb_sb[:], genb[:])
    nc.sync.dma_start(wscb_sb[:], wscb[:])
    nc.sync.dma_start(ident_sb[:], ident[:])
    nc.sync.dma_start(onesr_sb[:], onesr[:])

    # ---------------- hoist: pHT = i2h @ bH^T ----------------
    CB = 512
    with tc.tile_pool(name="hlhs", bufs=1) as hlhs, \
         tc.tile_pool(name="hrhs", bufs=2) as hrhs, \
         tc.tile_pool(name="hst", bufs=3) as hst, \
         tc.tile_pool(name="hps", bufs=4, space="PSUM") as hps:
        i2h_sb = hlhs.tile([128, DC * H], BF, name="i2h_sb")
        for k in range(DC):
            nc.sync.dma_start(i2h_sb[:, k * H:(k + 1) * H],
                              i2hT[k * 128:(k + 1) * 128, :])
        NCB = 2 * GRAN // CB
        for cb in range(NCB):
            rt = hrhs.tile([128, DC * CB], BF, name="h_rhs", tag="hrhs")
            in3 = bHT[:, cb * CB:(cb + 1) * CB].rearrange(
                "(k p) f -> p k f", p=128)
            out3 = rt[:].rearrange("p (k f) -> p k f", k=DC)
            nc.sync.dma_start(out3, in3)
            for c in range(HC):
                ps = hps.tile([128, CB], F32, name="h_ps", tag="hps")
                for k in range(DC):
                    nc.tensor.matmul(ps[:], i2h_sb[:, k * H + c * 128:k * H + (c + 1) * 128],
                                     rt[:, k * CB:(k + 1) * CB],
                                     start=(k == 0), stop=(k == DC - 1))
                st = hst.tile([128, CB], BF, name="h_st", tag="hst")
                if c % 2 == 0:
                    nc.scalar.copy(st[:], ps[:])
                else:
                    nc.vector.tensor_copy(st[:], ps[:])
                half, off = cb // (NCB // 2), (cb % (NCB // 2)) * CB
                nc.sync.dma_start(pHT[c, half, :, off:off + CB], st[:])

    # ---------------- scan pools ----------------
    zp = ctx.enter_context(tc.tile_pool(name="zp", bufs=3))       # pH granules
    bhp = ctx.enter_context(tc.tile_pool(name="bhp", bufs=2))     # bHC chunks
    smx = ctx.enter_context(tc.tile_pool(name="smx", bufs=1))     # softmax tiles
    lst = ctx.enter_context(tc.tile_pool(name="lst", bufs=1))     # LSTM temps
    ohp = ctx.enter_context(tc.tile_pool(name="ohp", bufs=2))     # onehot
    lop = ctx.enter_context(tc.tile_pool(name="lop", bufs=2))     # logits stage
    # (bHC is streamed per slot: tail ctx reads bf16 chunks, full precision)

    # PSUM banks (executor poisons a whole bank on group start, so in-flight
    # accumulators need private banks): eTA, eTB, trT, bankC(cx|lo|ph-tmp),
    # gA(2), gB(2) = 8 banks
    pps = ctx.enter_context(tc.tile_pool(name="pps", bufs=1, space="PSUM"))
    eTA = pps.tile([128, HB], F32, name="eTA")
    eTB = pps.tile([128, HB], F32, name="eTB")
    eT_ps = [eTA[:], eTB[:]]
    bankC = pps.tile([128, 448], F32, name="bankC")
    cx_ps = bankC[:, 0:256]            # shared A/B (evac'd before reuse)
    lo_ps = bankC[:, 256:320]          # shared A/B (2*HB wide)
    pht_tmp = [bankC[:, 384:416], bankC[:, 416:448]]
    g_ps = [pps.tile([128, 4 * 8 * HB], F32, name=f"g{h}") for h in range(2)]
    trT = pps.tile([128, 128], BF, name="trT")                    # transposes

    # initial state: h = 0, c = 0, ph = h2h_b
    for h in range(2):
        nc.vector.memset(hT_sb[h][:], 0.0)
        nc.vector.memset(cB_sb[h][:], 0.0)
        ph3 = phT_sb[h][:].rearrange("p (c b) -> p c b", b=HB)
        bb = h2hbT_sb[:].unsqueeze(2).broadcast_to((128, HC, HB))
        nc.vector.tensor_copy(ph3, bb)

    # granule DMA prefetch queue: linear over (slot, c)
    slot_list = [(s, X) for s in range(steps) for X in (0, 1)]
    gr_tiles = {}
    issued = [0]

    def ensure_issued(upto):
        while issued[0] <= upto and issued[0] < len(slot_list) * HC:
            k = issued[0]
            s_, X_ = slot_list[k // HC]
            c_ = k % HC
            g = zp.tile([128, GRAN], BF, name="g_t", tag="g")
            nc.sync.dma_start(g[:], pHT[c_, X_])
            gr_tiles[k] = g
            issued[0] += 1

    def softmax_block(hf):
        """eT -> alpha (bf16, transposed back to [t,b'])."""
        e1 = smx.tile([128, HB], BF, name="e1", tag="e1")
        nc.vector.tensor_copy(e1[:], eS_sb[hf][:])
        tp = trT[0:HB, :]
        nc.tensor.transpose(tp, e1[:], ident_sb[:])
        ex = smx.tile([HB, 128], F32, name="ex", tag="ex")
        nc.scalar.activation(ex[:], tp, EXP)
        sg = smx.tile([HB, 1], F32, name="sg", tag="sg")
        nc.vector.tensor_reduce(sg[:], ex[:], mybir.AxisListType.X, ADD)
        rc = smx.tile([HB, 1], F32, name="rc", tag="rc")
        nc.vector.reciprocal(rc[:], sg[:])
        al = smx.tile([HB, 128], BF, name="al", tag="al")
        nc.vector.tensor_scalar(al[:], ex[:], rc[:], None, MULT)
        ap = trT[:, 0:HB]
        nc.tensor.transpose(ap, al[:], ident_sb[0:HB, 0:HB])
        nc.vector.tensor_copy(alB_sb[hf][:], ap)

    def ctx_mms(hf):
        cxs = cx_ps
        for j in range(8):           # chunks of 4 b-rows (4*1024 cols bf16)
            bh = bhp.tile([T, 4 * D], BF, name="bh_t", tag="bh")
            col0 = (hf * HB + j * 4) * D
            nc.gpsimd.dma_start(bh[:], bHC[:, col0:col0 + 4 * D])
            for bs in range(4):
                b = j * 4 + bs
                for c in range(DC):
                    nc.tensor.matmul(cxs[:, c * HB + b:c * HB + b + 1],
                                     bh[:, bs * D + c * 128:bs * D + (c + 1) * 128],
                                     alB_sb[hf][:, b:b + 1], start=True, stop=True)

    def gates_ih(hf, s):
        """ctx evac + full gates: per-mc contiguous 26-matmul chains."""
        nc.vector.tensor_copy(ctxB_sb[hf][:], cx_ps)
        oh = ohp.tile([128, 2 * HB], BF, name="oh_t", tag="oh")
        nc.gpsimd.dma_start(oh[:], oneh[s, hf])
        gp = g_ps[hf]
        for mc in range(32):
            gsl = gp[:, mc * HB:(mc + 1) * HB]
            for k in range(HC):
                ms = slice(k * G4 + mc * 128, k * G4 + (mc + 1) * 128)
                nc.tensor.matmul(gsl, WhhHi_sb[:, ms],
                                 hT_sb[hf][:, k * HB:(k + 1) * HB],
                                 start=(k == 0), stop=False)
                nc.tensor.matmul(gsl, WhhLo_sb[:, ms],
                                 hT_sb[hf][:, k * HB:(k + 1) * HB],
                                 start=False, stop=False)
            for k in range(2):
                nc.tensor.matmul(gsl, Wtok_sb[:, k * G4 + mc * 128:k * G4 + (mc + 1) * 128],
                                 oh[:, k * HB:(k + 1) * HB],
                                 start=False, stop=False)
            for k in range(DC):
                ms = slice(k * G4 + mc * 128, k * G4 + (mc + 1) * 128)
                rhs = ctxB_sb[hf][:, k * HB:(k + 1) * HB]
                nc.tensor.matmul(gsl, WihHi_sb[:, ms], rhs,
                                 start=False, stop=False)
                nc.tensor.matmul(gsl, WihLo_sb[:, ms], rhs,
                                 start=False, stop=(k == DC - 1))

    def lstm_h2h_logits(hf, s, do_pregates):
        Q = HC * HB
        gp = g_ps[hf]
        ii = lst.tile([128, Q], BF, name="ii", tag="ii")
        ff = lst.tile([128, Q], BF, name="ff", tag="ff")
        gg = lst.tile([128, Q], BF, name="gg", tag="gg")
        oo = lst.tile([128, Q], BF, name="oo", tag="oo")
        nc.scalar.activation(ii[:], gp[:, 0 * Q:1 * Q], TANH, scale=0.5 / SW)
        nc.scalar.activation(ff[:], gp[:, 1 * Q:2 * Q], TANH, scale=0.5 / SW)
        nc.scalar.activation(gg[:], gp[:, 2 * Q:3 * Q], TANH, scale=1.0 / SW)
        nc.scalar.activation(oo[:], gp[:, 3 * Q:4 * Q], TANH, scale=0.5 / SW)
        nc.vector.tensor_scalar(ii[:], ii[:], 0.5, 0.5, MULT, ADD)
        nc.vector.tensor_scalar(ff[:], ff[:], 0.5, 0.5, MULT, ADD)
        nc.vector.tensor_scalar(oo[:], oo[:], 0.5, 0.5, MULT, ADD)
        nc.vector.tensor_tensor(ii[:], ii[:], gg[:], MULT)          # i*g~
        nc.vector.tensor_tensor(cB_sb[hf][:], cB_sb[hf][:], ff[:], MULT)
        nc.vector.tensor_tensor(cB_sb[hf][:], cB_sb[hf][:], ii[:], ADD)
        th = lst.tile([128, Q], BF, name="th", tag="th")
        nc.scalar.activation(th[:], cB_sb[hf][:], TANH)
        nc.vector.tensor_tensor(hT_sb[hf][:], oo[:], th[:], MULT)
        # h2h -> phT for next step (per-chunk psum tmp, bias via per-part scalar)
        for c in range(HC):
            psl = pht_tmp[c % 2]
            for k in range(HC):
                nc.tensor.matmul(psl, h2h_sb[:, k * H + c * 128:k * H + (c + 1) * 128],
                                 hT_sb[hf][:, k * HB:(k + 1) * HB],
                                 start=(k == 0), stop=(k == HC - 1),
                                 skip_group_check=True)
            nc.vector.tensor_scalar(phT_sb[hf][:, c * HB:(c + 1) * HB], psl,
                                    h2hbT_sb[:, c:c + 1], None, ADD)
        # logits
        los = lo_ps
        for vc in range(2):
            lsl = los[:, vc * HB:(vc + 1) * HB]
            for k in range(HC):
                nc.tensor.matmul(lsl, genT_sb[:, k * V + vc * 128:k * V + (vc + 1) * 128],
                                 hT_sb[hf][:, k * HB:(k + 1) * HB],
                                 start=(k == 0), stop=False, skip_group_check=True)
            nc.tensor.matmul(lsl, genb_sb[0:1, vc * 128:(vc + 1) * 128],
                             onesr_sb[0:1, :], start=False, stop=True,
                             skip_group_check=True)
        lo = lop.tile([128, 2 * HB], F32, name="lo_st", tag="lo")
        nc.vector.tensor_copy(lo[:], los)
        nc.gpsimd.dma_start(out[s, hf], lo[:])

    def attention(X, s, slot_idx, spl1, spl2):
        base = slot_idx * HC
        for c in range(HC):
            ensure_issued(base + c + 2)
            g = gr_tiles.pop(base + c)
            g3 = g[:].rearrange("p (t b) -> p t b", b=HB)
            phb = phT_sb[X][:, c * HB:(c + 1) * HB].unsqueeze(1) \
                .broadcast_to((128, T, HB))
            nc.vector.tensor_tensor(g3, g3, phb, ADD)
            nc.scalar.activation(g[:], g[:], TANH)
            gb = g[:].rearrange("p (t b) -> p b t", b=HB)
            for b in range(HB):
                nc.tensor.matmul(eT_ps[X][:, b:b + 1], gb[:, b, :],
                                 wscb_sb[:, c:c + 1], start=True, stop=True)
            if c == 0:
                nc.vector.tensor_copy(eS_sb[X][:], eT_ps[X])
            else:
                nc.vector.tensor_tensor(eS_sb[X][:], eS_sb[X][:], eT_ps[X], ADD)
            if c == 1 and spl1 is not None:
                spl1()
            if c == 3 and spl2 is not None:
                spl2()

    for s in range(steps):
        for X in (0, 1):
            Y, sy = 1 - X, (s if X == 1 else s - 1)
            slot_idx = s * 2 + X
            if sy >= 0:
                softmax_block(Y)
                ctx_mms(Y)
                attention(X, s, slot_idx,
                          spl1=lambda Y=Y, sy=sy: gates_ih(Y, sy),
                          spl2=lambda Y=Y, sy=sy: lstm_h2h_logits(
                              Y, sy, do_pregates=(sy + 1 < steps)))
            else:
                attention(X, s, slot_idx, None, None)
    # debug taps (steps==1 only): e1/alpha/ctx/h of stream 0
    if steps == 1:
        dbg = lop.tile([128, 2 * HB], F32, name="dbg", tag="lo")
        nc.vector.tensor_copy(dbg[:, 0:HB], eT_ps[0])
        nc.gpsimd.dma_start(out[10, 0, :, 0:HB], dbg[:, 0:HB])
        dbg2 = lop.tile([128, 2 * HB], F32, name="dbg2", tag="lo")
        nc.vector.tensor_copy(dbg2[:, 0:HB], alB_sb[0][:])
        nc.gpsimd.dma_start(out[11, 0, :, 0:HB], dbg2[:, 0:HB])
        dbg3 = lop.tile([128, 2 * HB], F32, name="dbg3", tag="lo")
        nc.vector.tensor_copy(dbg3[:], ctxB_sb[0][:, 0:2 * HB])
        nc.gpsimd.dma_start(out[12, 0], dbg3[:])
        dbg4 = lop.tile([128, 2 * HB], F32, name="dbg4", tag="lo")
        nc.vector.tensor_copy(dbg4[:], hT_sb[0][:, 0:2 * HB])
        nc.gpsimd.dma_start(out[13, 0], dbg4[:])
    # epilogue: finish stream B at the last step
    softmax_block(1)
    ctx_mms(1)
    gates_ih(1, steps - 1)
    lstm_h2h_logits(1, steps - 1, do_pregates=False)

    ctx.close()
    nc.compile()
    return nc


def _prep_core(ci, batch_H, text, shared):
    bH = batch_H[ci * BL:(ci + 1) * BL]          # [64, 128, 1024] f32
    tx = text[ci * BL:(ci + 1) * BL]             # [64, 26]
    # bHT [d, (half, t, b')]
    bHT = np.ascontiguousarray(
        bH.reshape(2, HB, T, D).transpose(3, 0, 2, 1).reshape(D, 2 * GRAN)
    ).astype(BF16)
    # bHC [t, (b, d)]
    bHC = np.ascontiguousarray(bH.transpose(1, 0, 2).reshape(T, BL * D)).astype(BF16)
    # onehot [26, 2, 128, (r 2, b' 32)] bf16
    oh = np.zeros((STEPS, 2, 128, 2 * HB), dtype=BF16)
    for s in range(STEPS):
        for half in range(2):
            for b in range(HB):
                v = int(tx[half * HB + b, s])
                oh[s, half, v % 128, (v // 128) * HB + b] = 1.0
    m = dict(shared)
    m.update({"bHT": bHT, "bHC": bHC, "oneh": oh})
    return m


def _chunkT(W, dt):
    # W [out, K] -> [128, (K/128 chunks, out)]
    K = W.shape[1]
    arr = W.T.reshape(K // 128, 128, W.shape[0]).transpose(1, 0, 2)
    return np.ascontiguousarray(arr.reshape(128, -1)).astype(dt)


def kernel(batch_H, text, i2h_w, h2h_w, h2h_b, score_w, W_ih, W_hh, b_ih, b_hh,
           gen_w, gen_b):
    batch_H = np.asarray(batch_H, dtype=np.float32)
    text = np.asarray(text)
    f32 = lambda x: np.asarray(x, dtype=np.float32)
    i2h_w, h2h_w, h2h_b = f32(i2h_w), f32(h2h_w), f32(h2h_b)
    score_w, W_ih, W_hh = f32(score_w), f32(W_ih), f32(W_hh)
    b_ih, b_hh, gen_w, gen_b = f32(b_ih), f32(b_hh), f32(gen_w), f32(gen_b)

    Wtok = W_ih[:, D:] + (b_ih + b_hh)[:, None]      # [4096, 256]
    Wic = W_ih[:, :D] * SW
    WicHi = Wic.astype(E4M3).astype(np.float32)
    WicLo = Wic - WicHi
    Whs = W_hh * SW
    WhsHi = Whs.astype(E4M3).astype(np.float32)
    WhsLo = Whs - WhsHi

    shared = {
        "i2hT": np.ascontiguousarray(i2h_w.T).astype(BF16),
        "WihHi": _chunkT(WicHi, E4M3),
        "WihLo": _chunkT(WicLo, E4M3),
        "WhhHi": _chunkT(WhsHi, E4M3),
        "WhhLo": _chunkT(WhsLo, E4M3),
        "WtokT": _chunkT(Wtok * SW, E4M3),
        "h2hT": _chunkT(h2h_w, E4M3),
        "h2hbT": np.ascontiguousarray(h2h_b.reshape(HC, 128).T).astype(np.float32),
        "genT": np.ascontiguousarray(
            gen_w.T.reshape(HC, 128, V).transpose(1, 0, 2).reshape(128, HC * V)
        ).astype(BF16),
        "genb": gen_b.reshape(1, V).astype(BF16),
        "wscb": np.ascontiguousarray(score_w[0].reshape(HC, 128).T).astype(BF16),
        "ident": np.eye(128, dtype=BF16),
        "onesr": np.ones((1, HB), dtype=BF16),
    }

    nc = build_kernel()
    in_maps = [_prep_core(ci, batch_H, text, shared) for ci in range(NC_)]
    import os
    do_trace = bool(int(os.environ.get("KERNEL_TRACE", "0")))
    res = run_bass_kernel_spmd(nc, in_maps, core_ids=list(range(NC_)),
                               trace=do_trace)
    global LAST_RESULT
    LAST_RESULT = res
    outs = res.results
    logits = np.zeros((B, STEPS, V), dtype=np.float32)
    for ci in range(NC_):
        o = outs[ci]["out"] if isinstance(outs[ci], dict) else outs[ci]
        # o [26, 2, 128, 2*32] -> logits[b, s, v]
        o = o.reshape(STEPS, 2, 128, 2, HB).transpose(1, 4, 0, 3, 2)
        logits[ci * BL:(ci + 1) * BL] = o.reshape(BL, STEPS, V)
    return logits


if __name__ == "__main__":
    np.random.seed(0)
    import reference
    inp = {k: np.asarray(v) for k, v in reference.setup_inputs().items()}
    got = kernel(**inp)
    exp = np.asarray(reference.reference(**inp))
    l2 = np.linalg.norm(got - exp) / np.linalg.norm(exp)
    print("l2 rel err:", l2)


# revision 17
# speedup vs baseline: 1.3439x; 1.2514x over previous
"""Trainium2 Bass kernel for nn_Attention (Bahdanau attention + LSTM scan).

Data-parallel over batch B=512 across 8 NeuronCores (64 rows/core).  Per core
the 64 rows are split into two 32-row half-batch streams (A/B) pipelined so
one stream's softmax/ctx/gates/LSTM tail hides inside the other stream's tanh
window (the ACT engine, at ~0.83 ns/elem-row, is the throughput bound).

Precision: matmul operands may mix dtypes (PE upconverts); all "moving" rhs
operands (alpha, ctx, h, onehot) stay bf16, stationary weights use fp8-e3m4
(W_ih with an extra e3m4 residual term), bHC fp8-e3m4, h2h fp8-e4m3,
logits/genT bf16.  pH is bf16, streamed from DRAM each step (z-add runs at
DVE 2x only for 2-byte dtypes).
"""
import sys
from contextlib import ExitStack

import numpy as np
import ml_dtypes

sys.path.insert(0, "/opt/trn_rl_repo")

import concourse.bass as bass  # noqa: E402
from concourse import bacc  # noqa: E402
import concourse.tile as tile  # noqa: E402
from concourse import mybir  # noqa: E402
from concourse.bass_utils import run_bass_kernel_spmd  # noqa: E402

BF16 = ml_dtypes.bfloat16
E3M4 = ml_dtypes.float8_e3m4
E4M3 = ml_dtypes.float8_e4m3
LAST_RESULT = None
BF = mybir.dt.bfloat16
F32 = mybir.dt.float32
E3 = mybir.dt.float8e3
E4 = mybir.dt.float8e4
ADD = mybir.AluOpType.add
MULT = mybir.AluOpType.mult
TANH = mybir.ActivationFunctionType.Tanh
EXP = mybir.ActivationFunctionType.Exp

B, T, D, H, V, STEPS = 512, 128, 1024, 1024, 256, 26
NC_ = 8
BL = B // NC_          # 64 batch rows per core
HB = BL // 2           # 32 rows per half-batch stream
HC = 8                 # 128-chunks of H
DC = 8                 # 128-chunks of D
G4 = 4 * H             # 4096
GRAN = T * HB          # 4096 cols per attention granule (t outer, b' inner)
SW = 128.0             # weight pre-scale before e4m3 quantization
SBHC = 16.0            # bHC pre-scale


def build_kernel(steps=STEPS):
    nc = bacc.Bacc("TRN2", target_bir_lowering=False, debug=False)
    # ---- inputs ----
    bHT = nc.declare_dram_parameter("bHT", [D, 2 * GRAN], BF, isOutput=False)
    bHC = nc.declare_dram_parameter("bHC", [T, BL * D], BF, isOutput=False)
    i2hT = nc.declare_dram_parameter("i2hT", [D, H], BF, isOutput=False)
    WihHi = nc.declare_dram_parameter("WihHi", [128, DC * G4], E4, isOutput=False)
    WihLo = nc.declare_dram_parameter("WihLo", [128, DC * G4], E4, isOutput=False)
    WhhHi = nc.declare_dram_parameter("WhhHi", [128, HC * G4], E4, isOutput=False)
    WhhLo = nc.declare_dram_parameter("WhhLo", [128, HC * G4], E4, isOutput=False)
    WtokT = nc.declare_dram_parameter("WtokT", [128, 2 * G4], E4, isOutput=False)
    h2hT = nc.declare_dram_parameter("h2hT", [128, HC * H], E4, isOutput=False)
    h2hbT = nc.declare_dram_parameter("h2hbT", [128, HC], F32, isOutput=False)
    genT = nc.declare_dram_parameter("genT", [128, HC * V], BF, isOutput=False)
    genb = nc.declare_dram_parameter("genb", [1, V], BF, isOutput=False)
    wscb = nc.declare_dram_parameter("wscb", [128, HC], BF, isOutput=False)
    oneh = nc.declare_dram_parameter("oneh", [STEPS, 2, 128, 2 * HB], BF, isOutput=False)
    ident = nc.declare_dram_parameter("ident", [128, 128], BF, isOutput=False)
    onesr = nc.declare_dram_parameter("onesr", [1, HB], BF, isOutput=False)
    out = nc.declare_dram_parameter("out", [STEPS, 2, 128, 2 * HB], F32, isOutput=True)
    pHT = nc.dram_tensor("pHT", [HC, 2, 128, GRAN], BF)

    ctx = ExitStack()
    tc = ctx.enter_context(tile.TileContext(nc))

    # ---------------- persistent SBUF ----------------
    res = ctx.enter_context(tc.tile_pool(name="res", bufs=1))
    WihHi_sb = res.tile([128, DC * G4], E4, name="WihHi_sb")   # 32 KiB/part
    WihLo_sb = res.tile([128, DC * G4], E4, name="WihLo_sb")   # 32 KiB
    WhhHi_sb = res.tile([128, HC * G4], E4, name="WhhHi_sb")   # 32 KiB
    WhhLo_sb = res.tile([128, HC * G4], E4, name="WhhLo_sb")   # 32 KiB
    Wtok_sb = res.tile([128, 2 * G4], E4, name="Wtok_sb")      # 8 KiB
    h2h_sb = res.tile([128, HC * H], E4, name="h2h_sb")        # 8 KiB
    h2hbT_sb = res.tile([128, HC], F32, name="h2hbT_sb")
    genT_sb = res.tile([128, HC * V], BF, name="genT_sb")      # 4 KiB
    genb_sb = res.tile([1, V], BF, name="genb_sb")
    wscb_sb = res.tile([128, HC], BF, name="wscb_sb")
    ident_sb = res.tile([128, 128], BF, name="ident_sb")
    onesr_sb = res.tile([1, HB], BF, name="onesr_sb")
    # per-stream state: layout [128, (hc 8, b' 32)]
    phT_sb = [res.tile([128, HC * HB], BF, name=f"phT{h}") for h in range(2)]
    hT_sb = [res.tile([128, HC * HB], BF, name=f"hT{h}") for h in range(2)]
    cB_sb = [res.tile([128, HC * HB], F32, name=f"cB{h}") for h in range(2)]
    ctxB_sb = [res.tile([128, DC * HB], BF, name=f"ctxB{h}") for h in range(2)]
    alB_sb = [res.tile([128, HB], BF, name=f"alB{h}") for h in range(2)]
    eS_sb = [res.tile([128, HB], F32, name=f"eS{h}") for h in range(2)]

    nc.sync.dma_start(WihHi_sb[:], WihHi[:])
    nc.sync.dma_start(WihLo_sb[:], WihLo[:])
    nc.sync.dma_start(WhhHi_sb[:], WhhHi[:])
    nc.sync.dma_start(WhhLo_sb[:], WhhLo[:])
    nc.sync.dma_start(Wtok_sb[:], WtokT[:])
    nc.sync.dma_start(h2h_sb[:], h2hT[:])
    nc.sync.dma_start(h2hbT_sb[:], h2hbT[:])
    nc.sync.dma_start(genT_sb[:], genT[:])
    nc.sync.dma_start(genb_sb[:], genb[:])
    nc.sync.dma_start(wscb_sb[:], wscb[:])
    nc.sync.dma_start(ident_sb[:], ident[:])
    nc.sync.dma_start(onesr_sb[:], onesr[:])

    # ---------------- hoist: pHT = i2h @ bH^T ----------------
    CB = 512
    with tc.tile_pool(name="hlhs", bufs=1) as hlhs, \
         tc.tile_pool(name="hrhs", bufs=2) as hrhs, \
         tc.tile_pool(name="hst", bufs=3) as hst, \
         tc.tile_pool(name="hps", bufs=4, space="PSUM") as hps:
        i2h_sb = hlhs.tile([128, DC * H], BF, name="i2h_sb")
        for k in range(DC):
            nc.sync.dma_start(i2h_sb[:, k * H:(k + 1) * H],
                              i2hT[k * 128:(k + 1) * 128, :])
        NCB = 2 * GRAN // CB
        for cb in range(NCB):
            rt = hrhs.tile([128, DC * CB], BF, name="h_rhs", tag="hrhs")
            in3 = bHT[:, cb * CB:(cb + 1) * CB].rearrange(
                "(k p) f -> p k f", p=128)
            out3 = rt[:].rearrange("p (k f) -> p k f", k=DC)
            nc.sync.dma_start(out3, in3)
            for c in range(HC):
                ps = hps.tile([128, CB], F32, name="h_ps", tag="hps")
                for k in range(DC):
                    nc.tensor.matmul(ps[:], i2h_sb[:, k * H + c * 128:k * H + (c + 1) * 128],
                                     rt[:, k * CB:(k + 1) * CB],
                                     start=(k == 0), stop=(k == DC - 1))
                st = hst.tile([128, CB], BF, name="h_st", tag="hst")
                if c % 2 == 0:
                    nc.scalar.copy(st[:], ps[:])
                else:
                    nc.vector.tensor_copy(st[:], ps[:])
                half, off = cb // (NCB // 2), (cb % (NCB // 2)) * CB
                nc.sync.dma_start(pHT[c, half, :, off:off + CB], st[:])

    # ---------------- scan pools ----------------
    zp = ctx.enter_context(tc.tile_pool(name="zp", bufs=3))       # pH granules
    bhp = ctx.enter_context(tc.tile_pool(name="bhp", bufs=2))     # bHC chunks
    smx = ctx.enter_context(tc.tile_pool(name="smx", bufs=1))     # softmax tiles
    lst = ctx.enter_context(tc.tile_pool(name="lst", bufs=1))     # LSTM temps
    ohp = ctx.enter_context(tc.tile_pool(name="ohp", bufs=2))     # onehot
    lop = ctx.enter_context(tc.tile_pool(name="lop", bufs=2))     # logits stage
    # (bHC is streamed per slot: tail ctx reads bf16 chunks, full precision)

    # PSUM banks (executor poisons a whole bank on group start, so in-flight
    # accumulators need private banks): eTA, eTB, trT, bankC(cx|lo|ph-tmp),
    # gA(2), gB(2) = 8 banks
    pps = ctx.enter_context(tc.tile_pool(name="pps", bufs=1, space="PSUM"))
    eTA = pps.tile([128, HB], F32, name="eTA")
    eTB = pps.tile([128, HB], F32, name="eTB")
    eT_ps = [eTA[:], eTB[:]]
    bankC = pps.tile([128, 448], F32, name="bankC")
    cx_ps = bankC[:, 0:256]            # shared A/B (evac'd before reuse)
    lo_ps = bankC[:, 256:320]          # shared A/B (2*HB wide)
    pht_tmp = [bankC[:, 384:416], bankC[:, 416:448]]
    g_ps = [pps.tile([128, 4 * 8 * HB], F32, name=f"g{h}") for h in range(2)]
    trT = pps.tile([128, 128], BF, name="trT")                    # transposes

    # initial state: h = 0, c = 0, ph = h2h_b
    for h in range(2):
        nc.vector.memset(hT_sb[h][:], 0.0)
        nc.vector.memset(cB_sb[h][:], 0.0)
        ph3 = phT_sb[h][:].rearrange("p (c b) -> p c b", b=HB)
        bb = h2hbT_sb[:].unsqueeze(2).broadcast_to((128, HC, HB))
        nc.vector.tensor_copy(ph3, bb)

    # granule DMA prefetch queue: linear over (slot, c)
    slot_list = [(s, X) for s in range(steps) for X in (0, 1)]
    gr_tiles = {}
    issued = [0]

    def ensure_issued(upto):
        while issued[0] <= upto and issued[0] < len(slot_list) * HC:
            k = issued[0]
            s_, X_ = slot_list[k // HC]
            c_ = k % HC
            g = zp.tile([128, GRAN], BF, name="g_t", tag="g")
            nc.sync.dma_start(g[:], pHT[c_, X_])
            gr_tiles[k] = g
            issued[0] += 1

    def softmax_block(hf):
        """eT -> alpha (bf16, transposed back to [t,b'])."""
        e1 = smx.tile([128, HB], BF, name="e1", tag="e1")
        nc.vector.tensor_copy(e1[:], eS_sb[hf][:])
        tp = trT[0:HB, :]
        nc.tensor.transpose(tp, e1[:], ident_sb[:])
        ex = smx.tile([HB, 128], F32, name="ex", tag="ex")
        nc.scalar.activation(ex[:], tp, EXP)
        sg = smx.tile([HB, 1], F32, name="sg", tag="sg")
        nc.vector.tensor_reduce(sg[:], ex[:], mybir.AxisListType.X, ADD)
        rc = smx.tile([HB, 1], F32, name="rc", tag="rc")
        nc.vector.reciprocal(rc[:], sg[:])
        al = smx.tile([HB, 128], BF, name="al", tag="al")
        nc.vector.tensor_scalar(al[:], ex[:], rc[:], None, MULT)
        ap = trT[:, 0:HB]
        nc.tensor.transpose(ap, al[:], ident_sb[0:HB, 0:HB])
        nc.vector.tensor_copy(alB_sb[hf][:], ap)

    def ctx_mms(hf):
        cxs = cx_ps
        for j in range(8):           # chunks of 4 b-rows (4*1024 cols bf16)
            bh = bhp.tile([T, 4 * D], BF, name="bh_t", tag="bh")
            col0 = (hf * HB + j * 4) * D
            nc.sync.dma_start(bh[:], bHC[:, col0:col0 + 4 * D])
            for bs in range(4):
                b = j * 4 + bs
                for c in range(DC):
                    nc.tensor.matmul(cxs[:, c * HB + b:c * HB + b + 1],
                                     bh[:, bs * D + c * 128:bs * D + (c + 1) * 128],
                                     alB_sb[hf][:, b:b + 1], start=True, stop=True)

    def gates_ih(hf, s):
        """ctx evac + full gates: per-mc contiguous 26-matmul chains."""
        nc.vector.tensor_copy(ctxB_sb[hf][:], cx_ps)
        oh = ohp.tile([128, 2 * HB], BF, name="oh_t", tag="oh")
        nc.gpsimd.dma_start(oh[:], oneh[s, hf])
        gp = g_ps[hf]
        for mc in range(32):
            gsl = gp[:, mc * HB:(mc + 1) * HB]
            for k in range(HC):
                ms = slice(k * G4 + mc * 128, k * G4 + (mc + 1) * 128)
                nc.tensor.matmul(gsl, WhhHi_sb[:, ms],
                                 hT_sb[hf][:, k * HB:(k + 1) * HB],
                                 start=(k == 0), stop=False)
                nc.tensor.matmul(gsl, WhhLo_sb[:, ms],
                                 hT_sb[hf][:, k * HB:(k + 1) * HB],
                                 start=False, stop=False)
            for k in range(2):
                nc.tensor.matmul(gsl, Wtok_sb[:, k * G4 + mc * 128:k * G4 + (mc + 1) * 128],
                                 oh[:, k * HB:(k + 1) * HB],
                                 start=False, stop=False)
            for k in range(DC):
                ms = slice(k * G4 + mc * 128, k * G4 + (mc + 1) * 128)
                rhs = ctxB_sb[hf][:, k * HB:(k + 1) * HB]
                nc.tensor.matmul(gsl, WihHi_sb[:, ms], rhs,
                                 start=False, stop=False)
                nc.tensor.matmul(gsl, WihLo_sb[:, ms], rhs,
                                 start=False, stop=(k == DC - 1))

    def lstm_h2h_logits(hf, s, do_pregates):
        Q = HC * HB
        gp = g_ps[hf]
        ii = lst.tile([128, Q], BF, name="ii", tag="ii")
        ff = lst.tile([128, Q], BF, name="ff", tag="ff")
        gg = lst.tile([128, Q], BF, name="gg", tag="gg")
        oo = lst.tile([128, Q], BF, name="oo", tag="oo")
        nc.scalar.activation(ii[:], gp[:, 0 * Q:1 * Q], TANH, scale=0.5 / SW)
        nc.scalar.activation(ff[:], gp[:, 1 * Q:2 * Q], TANH, scale=0.5 / SW)
        nc.scalar.activation(gg[:], gp[:, 2 * Q:3 * Q], TANH, scale=1.0 / SW)
        nc.scalar.activation(oo[:], gp[:, 3 * Q:4 * Q], TANH, scale=0.5 / SW)
        nc.vector.tensor_scalar(ii[:], ii[:], 0.5, 0.5, MULT, ADD)
        nc.vector.tensor_scalar(ff[:], ff[:], 0.5, 0.5, MULT, ADD)
        nc.vector.tensor_scalar(oo[:], oo[:], 0.5, 0.5, MULT, ADD)
        nc.vector.tensor_tensor(ii[:], ii[:], gg[:], MULT)          # i*g~
        nc.vector.tensor_tensor(cB_sb[hf][:], cB_sb[hf][:], ff[:], MULT)
        nc.vector.tensor_tensor(cB_sb[hf][:], cB_sb[hf][:], ii[:], ADD)
        th = lst.tile([128, Q], BF, name="th", tag="th")
        nc.scalar.activation(th[:], cB_sb[hf][:], TANH)
        nc.vector.tensor_tensor(hT_sb[hf][:], oo[:], th[:], MULT)
        # h2h -> phT for next step (per-chunk psum tmp, bias via per-part scalar)
        for c in range(HC):
            psl = pht_tmp[c % 2]
            for k in range(HC):
                nc.tensor.matmul(psl, h2h_sb[:, k * H + c * 128:k * H + (c + 1) * 128],
                                 hT_sb[hf][:, k * HB:(k + 1) * HB],
                                 start=(k == 0), stop=(k == HC - 1),
                                 skip_group_check=True)
            nc.vector.tensor_scalar(phT_sb[hf][:, c * HB:(c + 1) * HB], psl,
                                    h2hbT_sb[:, c:c + 1], None, ADD)
        # logits
        los = lo_ps
        for vc in range(2):
            lsl = los[:, vc * HB:(vc + 1) * HB]
            for k in range(HC):
                nc.tensor.matmul(lsl, genT_sb[:, k * V + vc * 128:k * V + (vc + 1) * 128],
                                 hT_sb[hf][:, k * HB:(k + 1) * HB],
                                 start=(k == 0), stop=False, skip_group_check=True)
            nc.tensor.matmul(lsl, genb_sb[0:1, vc * 128:(vc + 1) * 128],
                             onesr_sb[0:1, :], start=False, stop=True,
                             skip_group_check=True)
        lo = lop.tile([128, 2 * HB], F32, name="lo_st", tag="lo")
        nc.vector.tensor_copy(lo[:], los)
        nc.gpsimd.dma_start(out[s, hf], lo[:])

    def attention(X, s, slot_idx, spl1, spl2):
        base = slot_idx * HC
        for c in range(HC):
            ensure_issued(base + c + 2)
            g = gr_tiles.pop(base + c)
            g3 = g[:].rearrange("p (t b) -> p t b", b=HB)
            phb = phT_sb[X][:, c * HB:(c + 1) * HB].unsqueeze(1) \
                .broadcast_to((128, T, HB))
            nc.vector.tensor_tensor(g3, g3, phb, ADD)
            nc.scalar.activation(g[:], g[:], TANH)
            gb = g[:].rearrange("p (t b) -> p b t", b=HB)
            for b in range(HB):
                nc.tensor.matmul(eT_ps[X][:, b:b + 1], gb[:, b, :],
                                 wscb_sb[:, c:c + 1], start=True, stop=True)
            if c == 0:
                nc.vector.tensor_copy(eS_sb[X][:], eT_ps[X])
            else:
                nc.vector.tensor_tensor(eS_sb[X][:], eS_sb[X][:], eT_ps[X], ADD)
            if c == 1 and spl1 is not None:
                spl1()
            if c == 3 and spl2 is not None:
                spl2()

    for s in range(steps):
        for X in (0, 1):
            Y, sy = 1 - X, (s if X == 1 else s - 1)
            slot_idx = s * 2 + X
            if sy >= 0:
                softmax_block(Y)
                ctx_mms(Y)
                attention(X, s, slot_idx,
                          spl1=lambda Y=Y, sy=sy: gates_ih(Y, sy),
                          spl2=lambda Y=Y, sy=sy: lstm_h2h_logits(
                              Y, sy, do_pregates=(sy + 1 < steps)))
            else:
                attention(X, s, slot_idx, None, None)
    # debug taps (steps==1 only): e1/alpha/ctx/h of stream 0
    if steps == 1:
        dbg = lop.tile([128, 2 * HB], F32, name="dbg", tag="lo")
        nc.vector.tensor_copy(dbg[:, 0:HB], eT_ps[0])
        nc.gpsimd.dma_start(out[10, 0, :, 0:HB], dbg[:, 0:HB])
        dbg2 = lop.tile([128, 2 * HB], F32, name="dbg2", tag="lo")
        nc.vector.tensor_copy(dbg2[:, 0:HB], alB_sb[0][:])
        nc.gpsimd.dma_start(out[11, 0, :, 0:HB], dbg2[:, 0:HB])
        dbg3 = lop.tile([128, 2 * HB], F32, name="dbg3", tag="lo")
        nc.vector.tensor_copy(dbg3[:], ctxB_sb[0][:, 0:2 * HB])
        nc.gpsimd.dma_start(out[12, 0], dbg3[:])
        dbg4 = lop.tile([128, 2 * HB], F32, name="dbg4", tag="lo")
        nc.vector.tensor_copy(dbg4[:], hT_sb[0][:, 0:2 * HB])
        nc.gpsimd.dma_start(out[13, 0], dbg4[:])
    # epilogue: finish stream B at the last step
    softmax_block(1)
    ctx_mms(1)
    gates_ih(1, steps - 1)
    lstm_h2h_logits(1, steps - 1, do_pregates=False)

    ctx.close()
    nc.compile()
    return nc


def _prep_core(ci, batch_H, text, shared):
    bH = batch_H[ci * BL:(ci + 1) * BL]          # [64, 128, 1024] f32
    tx = text[ci * BL:(ci + 1) * BL]             # [64, 26]
    # bHT [d, (half, t, b')]
    bHT = np.ascontiguousarray(
        bH.reshape(2, HB, T, D).transpose(3, 0, 2, 1).reshape(D, 2 * GRAN)
    ).astype(BF16)
    # bHC [t, (b, d)]
    bHC = np.ascontiguousarray(bH.transpose(1, 0, 2).reshape(T, BL * D)).astype(BF16)
    # onehot [26, 2, 128, (r 2, b' 32)] bf16
    oh = np.zeros((STEPS, 2, 128, 2 * HB), dtype=BF16)
    for s in range(STEPS):
        for half in range(2):
            for b in range(HB):
                v = int(tx[half * HB + b, s])
                oh[s, half, v % 128, (v // 128) * HB + b] = 1.0
    m = dict(shared)
    m.update({"bHT": bHT, "bHC": bHC, "oneh": oh})
    return m


def _chunkT(W, dt):
    # W [out, K] -> [128, (K/128 chunks, out)]
    K = W.shape[1]
    arr = W.T.reshape(K // 128, 128, W.shape[0]).transpose(1, 0, 2)
    return np.ascontiguousarray(arr.reshape(128, -1)).astype(dt)


def kernel(batch_H, text, i2h_w, h2h_w, h2h_b, score_w, W_ih, W_hh, b_ih, b_hh,
           gen_w, gen_b):
    batch_H = np.asarray(batch_H, dtype=np.float32)
    text = np.asarray(text)
    f32 = lambda x: np.asarray(x, dtype=np.float32)
    i2h_w, h2h_w, h2h_b = f32(i2h_w), f32(h2h_w), f32(h2h_b)
    score_w, W_ih, W_hh = f32(score_w), f32(W_ih), f32(W_hh)
    b_ih, b_hh, gen_w, gen_b = f32(b_ih), f32(b_hh), f32(gen_w), f32(gen_b)

    Wtok = W_ih[:, D:] + (b_ih + b_hh)[:, None]      # [4096, 256]
    Wic = W_ih[:, :D] * SW
    WicHi = Wic.astype(E4M3).astype(np.float32)
    WicLo = Wic - WicHi
    Whs = W_hh * SW
    WhsHi = Whs.astype(E4M3).astype(np.float32)
    WhsLo = Whs - WhsHi

    shared = {
        "i2hT": np.ascontiguousarray(i2h_w.T).astype(BF16),
        "WihHi": _chunkT(WicHi, E4M3),
        "WihLo": _chunkT(WicLo, E4M3),
        "WhhHi": _chunkT(WhsHi, E4M3),
        "WhhLo": _chunkT(WhsLo, E4M3),
        "WtokT": _chunkT(Wtok * SW, E4M3),
        "h2hT": _chunkT(h2h_w, E4M3),
        "h2hbT": np.ascontiguousarray(h2h_b.reshape(HC, 128).T).astype(np.float32),
        "genT": np.ascontiguousarray(
            gen_w.T.reshape(HC, 128, V).transpose(1, 0, 2).reshape(128, HC * V)
        ).astype(BF16),
        "genb": gen_b.reshape(1, V).astype(BF16),
        "wscb": np.ascontiguousarray(score_w[0].reshape(HC, 128).T).astype(BF16),
        "ident": np.eye(128, dtype=BF16),
        "onesr": np.ones((1, HB), dtype=BF16),
    }

    nc = build_kernel()
    in_maps = [_prep_core(ci, batch_H, text, shared) for ci in range(NC_)]
    import os
    do_trace = bool(int(os.environ.get("KERNEL_TRACE", "0")))
    res = run_bass_kernel_spmd(nc, in_maps, core_ids=list(range(NC_)),
                               trace=do_trace)
    global LAST_RESULT
    LAST_RESULT = res
    outs = res.results
    logits = np.zeros((B, STEPS, V), dtype=np.float32)
    for ci in range(NC_):
        o = outs[ci]["out"] if isinstance(outs[ci], dict) else outs[ci]
        # o [26, 2, 128, 2*32] -> logits[b, s, v]
        o = o.reshape(STEPS, 2, 128, 2, HB).transpose(1, 4, 0, 3, 2)
        logits[ci * BL:(ci + 1) * BL] = o.reshape(BL, STEPS, V)
    return logits


if __name__ == "__main__":
    np.random.seed(0)
    import reference
    inp = {k: np.asarray(v) for k, v in reference.setup_inputs().items()}
    got = kernel(**inp)
    exp = np.asarray(reference.reference(**inp))
    l2 = np.linalg.norm(got - exp) / np.linalg.norm(exp)
    print("l2 rel err:", l2)
